# revision 98
# baseline (speedup 1.0000x reference)
"""Causal self-attention (RoPE) Trainium2 kernel — fp8 DoubleRow edition.

Full inputs -> shard across 8 NeuronCores (tensor-parallel over heads x
data-parallel over batch) -> bass/Tile kernel per core -> host partial-sum
unshard.

Reference semantics (B=2, T=2048, C=2048, H=16, hd=128):
    qkv = x @ w_qkv ; q,k,v split ; RoPE(q,k) ; causal softmax attention ;
    y = att @ v ; out = y @ w_proj
Core c handles batch b = c//4 and 4 heads h0 = 4*(c%4). Each core computes
out_partial[T, C] (bf16) = y_heads @ w_proj[rows of its heads]; the host sums
the 4 partials per batch in fp32.

Speed tricks vs the bf16 baseline:
- QKV / V / out projections run as fp8e4m3 DoubleRow matmuls (2 K=128
  products per 0.5-cycle-per-column instruction). Precision is preserved
  with a hi+lo split of both operands (3 of the 4 partial products are
  kept; the dropped lo*lo term is ~0.1%), so each K=128 chunk costs 0.75
  of its bf16 price. Host prepares x_hi/x_lo and w_hi/w_lo (w scaled by
  2^6 so the lo parts stay clear of the e4m3 subnormal floor; all three
  kept products share one power-of-2 psum scale, folded into the drains).
- PV is flipped (p chunk stationary, [v | 1/16] moving) so the softmax
  denominator accumulates for free in psum column 128, eliminating the
  separate ones-matmul rowsum. The 1/16 ones-column folds the 2^4 y
  quantization scale into the reciprocal.
- y [q, d] is normalized via per-partition activation scale, PE-transposed
  in bf16, then quantized to fp8 hi/lo for the DoubleRow out-projection.

Layout trick (baseline): RoPE pairs (2i, 2i+1) are permuted to (i, i+64) by
permuting w_qkv columns on the host, so rotate_half becomes a swap of the
top/bottom 64 partitions on the DVE. S is invariant since the same
orthogonal permutation is applied to q and k.
"""

import sys

sys.path.insert(0, "/opt/trn_rl_repo")

import numpy as np
import ml_dtypes

import concourse.bass as bass
import concourse.mybir as mybir
import concourse.tile as tile
from concourse import bacc, bass_utils

F32 = mybir.dt.float32
BF16 = mybir.dt.bfloat16
FP8 = mybir.dt.float8e4
E4M3 = ml_dtypes.float8_e4m3
DR = mybir.MatmulPerfMode.DoubleRow

T = 2048
C = 2048
HD = 128
NH = 16
NH_LOC = 4          # heads per core
N_CORES = 8
TQ = 512            # q-chunk (moving free dim)
KT = 128            # k-tile (S^T partition dim)
CK = 128            # contraction chunk over C
NCK = C // CK       # 16
NPAIR = NCK // 2    # 8 DoubleRow k-chunk pairs
NTQ = T // TQ       # 4
NKT = T // KT       # 16
SCALE = 1.0 / np.sqrt(HD)
W_SCALE = 64.0      # 2^6 weight prescale (keeps w_lo out of deep subnormals)
Y_SCALE = 16.0      # 2^4 y prescale, folded in via the 1/16 ones column

_compiled_nc = None
DEBUG_DUMPS = False


def _build():
    nc = bacc.Bacc("TRN2", target_bir_lowering=False, debug=False, num_devices=1)

    # pair-major packed layouts: one contiguous DMA per (chunk-pair)
    xq = nc.dram_tensor("xq", [NTQ, NPAIR, CK, 2, 2, TQ], FP8,
                        kind="ExternalInput").ap()
    wqk = nc.dram_tensor("wqk", [NPAIR, CK, 2, 2, 2 * NH_LOC * HD], FP8,
                         kind="ExternalInput").ap()
    wv = nc.dram_tensor("wv", [NPAIR, CK, 2, 2, NH_LOC * HD], FP8,
                        kind="ExternalInput").ap()
    wp = nc.dram_tensor("wp", [HD, 2, NH_LOC, C], FP8, kind="ExternalInput").ap()
    cosP = nc.dram_tensor("cosP", [HD, T], BF16, kind="ExternalInput").ap()
    sinP = nc.dram_tensor("sinP", [HD, T], BF16, kind="ExternalInput").ap()
    masks = nc.dram_tensor("masks", [KT, KT], BF16, kind="ExternalInput").ap()
    ident = nc.dram_tensor("ident", [128, 128], BF16, kind="ExternalInput").ap()
    out = nc.dram_tensor("out", [T, C], BF16, kind="ExternalOutput").ap()
    if DEBUG_DUMPS:
        dq = nc.dram_tensor("dq", [8 * HD, T], BF16, kind="ExternalOutput").ap()
        dv = nc.dram_tensor("dv", [NKT * KT, NH_LOC * (HD + 1)], BF16,
                            kind="ExternalOutput").ap()
        dyt = nc.dram_tensor("dyt", [HD, NKT * 2 * NH_LOC * KT], FP8,
                             kind="ExternalOutput").ap()

    with tile.TileContext(nc) as tc, (
        tc.tile_pool(name="persist", bufs=1)) as persist, (
        tc.tile_pool(name="weights", bufs=1)) as wpool, (
        tc.tile_pool(name="xstream", bufs=2)) as xstream, (
        tc.tile_pool(name="work", bufs=2)) as work, (
        tc.tile_pool(name="ps", bufs=1, space="PSUM")) as psp:

        # ---- persistent SBUF ----
        cos_sb = persist.tile([HD, T], BF16, tag="cos")
        sin_sb = persist.tile([HD, T], BF16, tag="sin")
        mask01_sb = persist.tile([KT, KT], BF16, tag="mask")
        id_sb = persist.tile([128, 128], BF16, tag="ident")

        # Q^T/K^T (d-major), V+ones (token-major), y^T fp8 hi/lo staging
        qk_sb = [persist.tile([HD, T], BF16, name=f"qk{m}", tag=f"qk{m}") for m in range(8)]
        v_sb = [persist.tile([KT, NH_LOC, HD + 1], BF16, name=f"v{i}", tag=f"v{i}") for i in range(NKT)]
        # [d, tok-tile, {hi,lo}, head, tok] — slot dim pairs either heads
        # (stride 128, at fixed hl) or hi/lo of one head (stride 512)
        yt_sb = persist.tile([HD, NKT, 2, NH_LOC, KT], FP8, tag="yt")
        for i in range(NKT):
            nc.vector.memset(v_sb[i][:, :, HD:HD + 1], 1.0 / Y_SCALE)

        # ---- weights (whole kernel lifetime) ----
        # pair tiles: [128, {kc even, kc odd}, {hi, lo}, cols]
        wqk_p = [wpool.tile([CK, 2, 2, 2 * NH_LOC * HD], FP8, name=f"wqk{i}", tag=f"wqk{i}") for i in range(NPAIR)]
        wv_p = [wpool.tile([CK, 2, 2, NH_LOC * HD], FP8, name=f"wv{i}", tag=f"wv{i}") for i in range(NPAIR)]
        # [d, {lo, hi}, head, c-cols]
        wp_sb = wpool.tile([HD, 2, NH_LOC, C], FP8, tag="wp")
        # x pair tiles: [128, {kc even, kc odd}, {lo, hi}, tq]
        xc0 = [xstream.tile([CK, 2, 2, TQ], FP8, name=f"xp{i}", tag=f"xp{i}") for i in range(NPAIR)]
        # Startup DMAs on the SP HWDGE queue in consumption order
        for i in range(NPAIR):
            nc.sync.dma_start(wqk_p[i][:], wqk[i])
            nc.sync.dma_start(xc0[i][:], xq[0, i])
        nc.sync.dma_start(cos_sb[:], cosP)
        nc.sync.dma_start(sin_sb[:], sinP)
        for i in range(NPAIR):
            nc.sync.dma_start(wv_p[i][:], wv[i])
        nc.sync.dma_start(mask01_sb[:], masks)
        nc.sync.dma_start(id_sb[:], ident)
        nc.sync.dma_start(wp_sb[:], wp)

        # PSUM tags (8 banks total, shared across phases via slot cycling).
        # NOTE: start=True zeroes the WHOLE bank (pending-zero is
        # bank-granular), so every accumulation group owns a full bank:
        #   "a": QKV psums m=0..3 / S^T tiles + y transposes / V psums /
        #        out-proj psums                                           (4)
        #   "b": QKV psums m=4,5 / flipped-PV accumulators s=0,1          (2)
        #   "c": QKV psum m=6 / flipped-PV accumulator s=2                (1)
        #   "d": QKV psum m=7 / flipped-PV accumulator s=3                (1)
        def ps_a():
            return psp.tile([128, TQ], F32, name="psa", tag="a", bufs=4)
        def ps_b():
            return psp.tile([128, TQ], F32, name="psb", tag="b", bufs=2)
        def ps_c():
            return psp.tile([128, TQ], F32, name="psc", tag="c", bufs=1)
        def ps_d():
            return psp.tile([128, TQ], F32, name="psd", tag="d", bufs=1)
        def ps_a_t():
            # transpose staging; same banks as "a" (2KB/partition each)
            return psp.tile([128, 1024], BF16, name="pst", tag="a", bufs=4)

        # warmup: keep the PE busy through the startup DMA window so the
        # p-state ramp (2.4GHz after 3us continuous busy) is paid for free.
        # memset on the (idle) GPSIMD so the DVE's v-ones memsets don't
        # delay the first warmup matmul.
        warm = work.tile([128, TQ], BF16, tag="warm", bufs=1)
        nc.gpsimd.memset(warm[:], 0.0)
        wps = psp.tile([128, TQ], F32, name="wps", tag="a", bufs=4)
        for _ in range(14):
            nc.tensor.matmul(wps[0:16, 0:256], warm[:, 0:16], warm[:, 0:256],
                             start=True, stop=True)

        def qkv_phase(jq, xp, skip=()):
            """Hi-lo fp8 DoubleRow QKV + V projection for token chunk jq
            (pair-outer so all psums progress as x/w tiles land), plus
            next-x prefetch. Products carry scale 2^6 (from w); drains
            scale by 2^-6."""
            tsl = slice(jq * TQ, (jq + 1) * TQ)
            mk = [ps_a] * 4 + [ps_b] * 2 + [ps_c, ps_d]
            qk_ps = [None if m in skip else mk[m]() for m in range(8)]
            # cross terms first within each pair: cross(2i) only needs
            # chunk 2i's DMAs, so the PE can start one transfer earlier
            for i in range(NPAIR):
                # cross: w_hi x x_lo + w_lo x x_hi, per k-chunk
                for j in range(2):
                    for m in range(8):
                        if qk_ps[m] is None:
                            continue
                        nc.tensor.matmul(
                            qk_ps[m][:],
                            wqk_p[i][:, j, :, m * 128:(m + 1) * 128],
                            xp[i][:, j, :, :],
                            start=(i == 0 and j == 0), stop=False, perf_mode=DR,
                        )
                # term1: w_hi (pair) x x_hi (pair)
                last = (i == NPAIR - 1)
                for m in range(8):
                    if qk_ps[m] is None:
                        continue
                    nc.tensor.matmul(
                        qk_ps[m][:],
                        wqk_p[i][:, :, 0, m * 128:(m + 1) * 128],
                        xp[i][:, :, 1, :],
                        start=False, stop=last, perf_mode=DR,
                    )
            # drain q/k of head h adjacently (0,4 then 1,5 ...) so the
            # attention phase's first S matmuls unblock after two drains
            for m in (0, 4, 1, 5, 2, 6, 3, 7):
                if qk_ps[m] is None:
                    continue
                dst = qk_sb[m][:, tsl]
                nc.scalar.activation(dst, qk_ps[m][:],
                                     mybir.ActivationFunctionType.Copy,
                                     scale=1.0 / W_SCALE)
                # RoPE in place on DVE: tmp = swap(dst)*sinSwap computed with
                # partition-aligned inputs and a shifted output partition.
                tmp = work.tile([HD, TQ], BF16, tag="rope", bufs=3)
                nc.vector.tensor_mul(tmp[0:64, :], dst[64:128, :], sin_sb[64:128, tsl])
                nc.vector.tensor_mul(tmp[64:128, :], dst[0:64, :], sin_sb[0:64, tsl])
                nc.vector.tensor_mul(dst, dst, cos_sb[:, tsl])
                nc.vector.tensor_add(dst, dst, tmp[:])

            # V projection, s4-outer: each token-subtile psum completes
            # and drains early, freeing its "a" slot before the phase
            # boundary (the pair-major packed wv DMAs land well before the
            # jq0 V phase starts, so this is safe for jq0 too).
            if True:
                for s4 in range(4):
                    # last subtile goes to bank "d" so its late drain does
                    # not gate the attention phase's first S matmul
                    v_ps = (psp.tile([128, 4, KT], F32, name="psv", tag="d",
                                     bufs=1) if s4 == 3 else
                            psp.tile([128, 4, KT], F32, name="psv", tag="a",
                                     bufs=4))
                    for i in range(NPAIR):
                        for j in range(2):
                            nc.tensor.matmul(
                                v_ps[:],
                                xp[i][:, j, :, s4 * KT:(s4 + 1) * KT],
                                wv_p[i][:, j, :, :],
                                start=(i == 0 and j == 0), stop=False,
                                perf_mode=DR,
                            )
                        nc.tensor.matmul(
                            v_ps[:],
                            xp[i][:, :, 1, s4 * KT:(s4 + 1) * KT],
                            wv_p[i][:, :, 0, :],
                            start=False, stop=(i == NPAIR - 1), perf_mode=DR,
                        )
                    nc.scalar.activation(
                        v_sb[jq * 4 + s4][:, :, 0:HD], v_ps[:],
                        mybir.ActivationFunctionType.Copy,
                        scale=1.0 / W_SCALE)

            # prefetch next x chunk right after its predecessor is consumed
            if jq + 1 < NTQ:
                xp_next = [xstream.tile([CK, 2, 2, TQ], FP8, name=f"xp{i}", tag=f"xp{i}") for i in range(NPAIR)]
                for i in range(NPAIR):
                    nc.sync.dma_start(xp_next[i][:], xq[jq + 1, i])
            else:
                xp_next = None
            return xp_next

        # out-projection tile: 6 DoubleRow matmuls + drain + DMA, split in
        # two filler-granular halves sharing one psum group.
        # products carry scale 2^4 (y) * 2^6 (wp) -> drain by 2^-10
        def op_tile_thunks(jqo, s4, cc, use_act, tail=False, split=False):
            it = jqo * 4 + s4
            csl = slice(cc * TQ, (cc + 1) * TQ)
            st = {}

            def part1():
                ps = st["ps"] = ps_a()
                for hp in range(2):
                    nc.tensor.matmul(
                        ps[:],
                        yt_sb[:, it, 0, 2 * hp:2 * hp + 2, :],
                        wp_sb[:, 1, 2 * hp:2 * hp + 2, csl],
                        start=(hp == 0), stop=False, perf_mode=DR,
                    )
                nc.tensor.matmul(
                    ps[:], yt_sb[:, it, :, 0, :], wp_sb[:, :, 0, csl],
                    start=False, stop=False, perf_mode=DR,
                )

            def part2():
                ps = st["ps"]
                for h2 in range(1, NH_LOC):
                    nc.tensor.matmul(
                        ps[:],
                        yt_sb[:, it, :, h2, :],
                        wp_sb[:, :, h2, csl],
                        start=False, stop=(h2 == NH_LOC - 1), perf_mode=DR,
                    )
                ot = work.tile([128, TQ], BF16, tag="ot", bufs=8)
                if tail:
                    # final tiles: fan the drains and DMA issues across all
                    # engine queues to shrink the end-of-kernel flush
                    if cc % 2 == 0:
                        nc.scalar.activation(
                            ot[:], ps[:], mybir.ActivationFunctionType.Copy,
                            scale=1.0 / (W_SCALE * Y_SCALE),
                        )
                    else:
                        nc.vector.tensor_scalar_mul(
                            ot[:], ps[:], 1.0 / (W_SCALE * Y_SCALE))
                    dma_eng = [nc.scalar, nc.sync, nc.gpsimd, nc.sync][cc % 4]
                    dma_eng.dma_start(out[it * KT:(it + 1) * KT, csl], ot[:])
                elif use_act:
                    nc.scalar.activation(
                        ot[:], ps[:], mybir.ActivationFunctionType.Copy,
                        scale=1.0 / (W_SCALE * Y_SCALE),
                    )
                    nc.scalar.dma_start(out[it * KT:(it + 1) * KT, csl], ot[:])
                else:
                    nc.vector.tensor_scalar_mul(
                        ot[:], ps[:], 1.0 / (W_SCALE * Y_SCALE))
                    nc.sync.dma_start(out[it * KT:(it + 1) * KT, csl], ot[:])

            return [part1, part2]

        def emit_op_tile(jqo, s4, cc, use_act, tail=False, split=False):
            for t in op_tile_thunks(jqo, s4, cc, use_act, tail=tail,
                                    split=split):
                t()

        # out-proj(jq) tiles are emitted as FILLER inside attention(jq+1):
        # per-tile exp on the activation engine outpaces the PE's S+PV work
        # there, so the PE interleaves independent out-proj matmuls into
        # the exp-serialization gaps.
        pending_ops = []

        def pull_ops(n=1):
            for _ in range(n):
                if pending_ops:
                    pending_ops.pop(0)()

        def qkv_m_thunks(jq1, xp1, m):
            """Deferred QKV m-group of chunk jq1, as filler thunks."""
            tsl1 = slice(jq1 * TQ, (jq1 + 1) * TQ)
            mcols = slice(m * 128, (m + 1) * 128)
            st = {}

            def pairs(i0, i1, first):
                ps = st["ps"]
                for i in range(i0, i1):
                    for j in range(2):
                        nc.tensor.matmul(
                            ps[:], wqk_p[i][:, j, :, mcols], xp1[i][:, j, :, :],
                            start=(first and i == i0 and j == 0), stop=False,
                            perf_mode=DR)
                    nc.tensor.matmul(
                        ps[:], wqk_p[i][:, :, 0, mcols], xp1[i][:, :, 1, :],
                        start=False, stop=(i == NPAIR - 1), perf_mode=DR)

            def t1():
                st["ps"] = ps_a()
                pairs(0, 3, True)

            def t2():
                pairs(3, 6, False)

            def t3():
                pairs(6, NPAIR, False)

            def t4():
                dst = qk_sb[m][:, tsl1]
                nc.scalar.activation(dst, st["ps"][:],
                                     mybir.ActivationFunctionType.Copy,
                                     scale=1.0 / W_SCALE)
                tmp = work.tile([HD, TQ], BF16, tag="rope", bufs=3)
                nc.vector.tensor_mul(tmp[0:64, :], dst[64:128, :], sin_sb[64:128, tsl1])
                nc.vector.tensor_mul(tmp[64:128, :], dst[0:64, :], sin_sb[0:64, tsl1])
                nc.vector.tensor_mul(dst, dst, cos_sb[:, tsl1])
                nc.vector.tensor_add(dst, dst, tmp[:])

            return [t1, t2, t3, t4]

        xp_sb = xc0
        for jq in range(NTQ):
            tsl = slice(jq * TQ, (jq + 1) * TQ)

            xp_next = qkv_phase(jq, xp_sb, skip=((7,) if jq == 1 else ()))
            if jq == 0:
                # attention(0) has no out-proj filler yet; feed it the
                # deferred m=7 QKV group of chunk 1
                pending_ops = qkv_m_thunks(1, xp_next, 7)

            # ======== attention for q-chunk jq, all local heads ========
            nk = 4 * jq + 4
            pull_every = max(1, (4 * nk) // 18)
            pv_count = 0
            pull_ops(7)  # cover the qk-drain/RoPE wait before the first S
            tp_tag = [("b", 2), ("b", 2), ("c", 1), ("d", 1)]
            for h in range(NH_LOC):
                # flipped-PV accumulators: [q, {y cols 0..127, rowsum/16}],
                # one full psum bank per accumulation group
                pv_ps = [
                    psp.tile([128, HD + 1], F32, name="pvb0", tag="b", bufs=2),
                    psp.tile([128, HD + 1], F32, name="pvb1", tag="b", bufs=2),
                    psp.tile([128, HD + 1], F32, name="pvc", tag="c", bufs=1),
                    psp.tile([128, HD + 1], F32, name="pvd", tag="d", bufs=1),
                ]

                # software-pipelined by one tile: S(ik+1) is issued before
                # PV(ik) so the PE has work while exp(ik) runs
                def s_exp(ik):
                    r = ik - 4 * jq
                    # columns q < 128*r of this S^T tile are fully masked
                    qo = 128 * r if r >= 1 else 0
                    diag = r >= 0
                    s_ps = ps_a()
                    nc.tensor.matmul(
                        s_ps[:, qo:],
                        qk_sb[4 + h][:, ik * KT:(ik + 1) * KT],
                        qk_sb[h][:, jq * TQ + qo:(jq + 1) * TQ],
                        start=True,
                        stop=not diag,
                    )
                    if diag:
                        # causal mask folded into the logits on the PE:
                        # adds -1e5 where k > q in the straddling 128-block
                        # (maskM.T @ I), so exp underflows to exact zeros and
                        # no masking sits on the exp->PV chain
                        nc.tensor.matmul(
                            s_ps[:, qo:qo + KT], mask01_sb[:], id_sb[:],
                            start=False, stop=True,
                        )
                    p_t = work.tile([KT, TQ], BF16, tag="p", bufs=8)
                    nc.scalar.activation(
                        p_t[:, qo:], s_ps[:, qo:],
                        mybir.ActivationFunctionType.Exp,
                        scale=float(SCALE),
                    )
                    return p_t, qo

                rsum = work.tile([128, 4], F32, tag="rsum", bufs=2)

                def pv(ik, p_t, qo):
                    r = ik - 4 * jq
                    ss = list(range(max(r, 0), 4))
                    for s in ss:
                        s_abs = 4 * jq + s
                        nc.tensor.matmul(
                            pv_ps[s][:],
                            p_t[:, s * KT:(s + 1) * KT],
                            v_sb[ik][:, h, :],
                            start=(ik == 0),
                            stop=(ik == s_abs),
                        )
                        if ik == s_abs:
                            # stage the rowsum copy as soon as this
                            # subtile's accumulation closes, ahead of the
                            # recip -> normalize -> transpose chain
                            nc.vector.tensor_copy(
                                rsum[:, s:s + 1], pv_ps[s][:, HD:HD + 1])

                def maybe_pull():
                    nonlocal pv_count
                    pv_count += 1
                    if pv_count % pull_every == 0:
                        pull_ops(1)

                # depth-2 software pipeline: two S/exp tiles in flight so
                # the exp ack/semaphore latency never reaches the PE
                depth = min(2, nk)
                pend = [s_exp(ik) for ik in range(depth)]
                for ik in range(depth, nk):
                    nxt = s_exp(ik)
                    # filler goes BEFORE pv: pv blocks the in-order PE
                    # queue on exp, filler does not
                    maybe_pull()
                    pv(ik - depth, *pend[0])
                    pend = pend[1:] + [nxt]
                for z, ik in enumerate(range(nk - depth, nk)):
                    maybe_pull()
                    pv(ik, *pend[z])

                # normalize (x16) on DVE, PE-transpose via the freed PV
                # banks, quantize hi/lo fp8 on DVE (act keeps only exp)
                recip = work.tile([128, 4], F32, tag="recip", bufs=2)
                nc.vector.reciprocal_approx_fast(recip[:], rsum[:])
                yns = []
                for s in range(4):
                    yn = work.tile([128, KT], BF16, tag="yn", bufs=4)
                    nc.vector.tensor_scalar(
                        yn[:], pv_ps[s][:, 0:HD], recip[:, s:s + 1], None,
                        mybir.AluOpType.mult)
                    yns.append(yn)
                pull_ops(2)  # cover the recip/normalize latency
                for s in range(4):
                    it = jq * 4 + s
                    tag, nb = tp_tag[s]
                    tp = psp.tile([128, 1024], BF16, name="tp", tag=tag, bufs=nb)
                    nc.tensor.transpose(tp[:, 0:KT], yns[s][:], id_sb[:])
                    nc.vector.tensor_copy(yt_sb[:, it, 0, h, :], tp[:, 0:KT])
                    nc.vector.tensor_tensor(yt_sb[:, it, 1, h, :], tp[:, 0:KT],
                                            yt_sb[:, it, 0, h, :],
                                            mybir.AluOpType.subtract)
                    if jq == NTQ - 1 and h == NH_LOC - 1:
                        # final chunk: out-proj inline as soon as the last
                        # head's subtile lands (shrinks the DMA tail)
                        for cc in range(4):
                            emit_op_tile(jq, s, cc, use_act=(cc % 2 == 0),
                                         tail=True)

            pull_ops(32)  # flush any leftover out-proj tiles of jq-1

            if jq < NTQ - 1:
                pending_ops = [
                    t
                    for s4 in range(4) for cc in range(4)
                    for t in op_tile_thunks(jq, s4, cc, use_act=False)
                ]

            xp_sb = xp_next

        if DEBUG_DUMPS:
            for m in range(8):
                nc.sync.dma_start(dq[m * HD:(m + 1) * HD, :], qk_sb[m][:])
            for i in range(NKT):
                nc.sync.dma_start(dv[i * KT:(i + 1) * KT, :], v_sb[i][:])
            nc.sync.dma_start(dyt, yt_sb[:])

    nc.compile()
    return nc


def _get_nc():
    global _compiled_nc
    if _compiled_nc is None:
        _compiled_nc = _build()
    return _compiled_nc


def _rope_tables():
    t = np.arange(T, dtype=np.float64)
    inv_freq = 1.0 / (10000.0 ** (np.arange(0, HD, 2, dtype=np.float64) / HD))
    freqs = np.outer(t, inv_freq)            # [T, 64]
    cos_half = np.cos(freqs).T               # [64, T]
    sin_half = np.sin(freqs).T
    cosP = np.concatenate([cos_half, cos_half], axis=0)      # [128, T]
    # tmp[j]    = dst[j+64] * sinSwap[j+64]  (= -sin_half[j])
    # tmp[j+64] = dst[j]    * sinSwap[j]     (= +sin_half[j])
    sinSwap = np.concatenate([sin_half, -sin_half], axis=0)
    return (cosP.astype(ml_dtypes.bfloat16), sinSwap.astype(ml_dtypes.bfloat16))


def _mask_tiles():
    # maskM[j, k] = -1e5 where k > j: (maskM.T @ I)[k, q] masks k > q
    j = np.arange(KT)[:, None]               # [128, 1]
    k = np.arange(KT)[None, :]               # [1, 128]
    return np.where(k > j, -1.0e5, 0.0).astype(ml_dtypes.bfloat16)


def _head_perm(h0):
    """Permuted q/k columns for heads h0..h0+3: pairs (2i,2i+1)->(i,i+64)."""
    cols = []
    for h in range(h0, h0 + NH_LOC):
        base = h * HD
        cols.extend(base + 2 * np.arange(64))
        cols.extend(base + 2 * np.arange(64) + 1)
    return np.array(cols)


def _hilo(a, scale):
    """(hi, lo) e4m3 split of a*scale (lo captures the rounding residual)."""
    s = (np.asarray(a, dtype=np.float32) * np.float32(scale))
    hi = s.astype(E4M3)
    lo = (s - hi.astype(np.float32)).astype(E4M3)
    return hi, lo


def _make_in_maps(x, w_qkv, w_proj):
    x = np.asarray(x)
    w_qkv = np.asarray(w_qkv)
    w_proj = np.asarray(w_proj)
    B = x.shape[0]
    assert x.shape == (B, T, C) and B == 2

    cosP, sinP = _rope_tables()
    masks = _mask_tiles()
    ident = np.eye(128, dtype=ml_dtypes.bfloat16)

    xq_b = []
    for b in range(B):
        xT = np.ascontiguousarray(x[b].T)
        hi, lo = _hilo(xT, 1.0)
        xs = np.stack([lo, hi], axis=1)            # [C, 2, T]
        # -> [NTQ, NPAIR, 128, {kc in pair}, {lo,hi}, TQ] pair-major pack
        xs = xs.reshape(NPAIR, 2, CK, 2, NTQ, TQ).transpose(4, 0, 2, 1, 3, 5)
        xq_b.append(np.ascontiguousarray(xs))

    in_maps = []
    for c in range(N_CORES):
        b = c // 4
        h0 = NH_LOC * (c % 4)
        perm = _head_perm(h0)
        def _pack_pairs(a):
            # [C, 2, W] -> [NPAIR, 128, {kc in pair}, {hi,lo}, W]
            W = a.shape[-1]
            return np.ascontiguousarray(
                a.reshape(NPAIR, 2, CK, 2, W).transpose(0, 2, 1, 3, 4))

        wqk_c = np.concatenate(
            [w_qkv[:, perm], w_qkv[:, C + perm]], axis=1
        )                                                   # [C, 1024]
        hi, lo = _hilo(wqk_c, W_SCALE)
        wqk_8 = _pack_pairs(np.stack([hi, lo], axis=1))
        vcols = np.arange(h0 * HD, (h0 + NH_LOC) * HD)
        hi, lo = _hilo(w_qkv[:, 2 * C + vcols], W_SCALE)
        wv_8 = _pack_pairs(np.stack([hi, lo], axis=1))
        wp_c = w_proj[h0 * HD:(h0 + NH_LOC) * HD, :]              # [512, C]
        wp_d = wp_c.reshape(NH_LOC, HD, C).transpose(1, 0, 2)     # [128,4,C]
        hi, lo = _hilo(wp_d, W_SCALE)
        wp_8 = np.ascontiguousarray(np.stack([lo, hi], axis=1))   # [128,2,4,C]
        in_maps.append({
            "xq": xq_b[b],
            "wqk": wqk_8,
            "wv": wv_8,
            "wp": wp_8,
            "cosP": cosP,
            "sinP": sinP,
            "masks": masks,
            "ident": ident,
        })
    return in_maps


def _reduce_out(results):
    out = np.zeros((2, T, C), dtype=np.float32)
    for c in range(N_CORES):
        out[c // 4] += results[c]["out"].astype(np.float32)
    return out


_cached_exec = None


def _get_cached_exec():
    """Build (once) a jitted SPMD executable for the compiled Bass module."""
    global _cached_exec
    if _cached_exec is not None:
        return _cached_exec
    import jax
    from jax.experimental.shard_map import shard_map
    from jax.sharding import Mesh, PartitionSpec
    from concourse import bass2jax

    nc = _get_nc()
    bass2jax.install_neuronx_cc_hook()
    partition_name = nc.partition_id_tensor.name if nc.partition_id_tensor else None
    in_names, out_names, out_avals = [], [], []
    for alloc in nc.m.functions[0].allocations:
        if not isinstance(alloc, mybir.MemoryLocationSet):
            continue
        name = alloc.memorylocations[0].name
        if alloc.kind == "ExternalInput":
            if name != partition_name:
                in_names.append(name)
        elif alloc.kind == "ExternalOutput":
            out_names.append(name)
            out_avals.append(
                jax.core.ShapedArray(
                    tuple(alloc.tensor_shape), mybir.dt.np(alloc.dtype)
                )
            )
    n_params = len(in_names)
    all_names = (
        tuple(in_names) + tuple(out_names)
        + ((partition_name,) if partition_name else ())
    )
    donate = tuple(range(n_params, n_params + len(out_names)))

    def _body(*args):
        operands = list(args)
        if partition_name is not None:
            operands.append(bass2jax.partition_id_tensor())
        outs = bass2jax._bass_exec_p.bind(
            *operands,
            out_avals=tuple(out_avals),
            in_names=all_names,
            out_names=tuple(out_names),
            lowering_input_output_aliases=(),
            sim_require_finite=True,
            sim_require_nnan=True,
            nc=nc,
        )
        return tuple(outs)

    devices = jax.devices()[:N_CORES]
    mesh = Mesh(np.asarray(devices), ("core",))
    nin = n_params + len(out_names)
    sharded = jax.jit(
        shard_map(
            _body,
            mesh=mesh,
            in_specs=(PartitionSpec("core"),) * nin,
            out_specs=(PartitionSpec("core"),) * len(out_names),
            check_rep=False,
        ),
        donate_argnums=donate,
        keep_unused=True,
    )
    _cached_exec = (sharded, in_names, out_names, out_avals)
    return _cached_exec


def _run_cached(in_maps):
    sharded, in_names, out_names, out_avals = _get_cached_exec()
    concat_in = [
        np.concatenate([np.asarray(in_maps[c][k]) for c in range(N_CORES)], axis=0)
        for k in in_names
    ]
    concat_zeros = [
        np.zeros((N_CORES * av.shape[0], *av.shape[1:]), av.dtype)
        for av in out_avals
    ]
    out_arrs = sharded(*concat_in, *concat_zeros)
    return [
        {
            k: np.asarray(out_arrs[i]).reshape(N_CORES, *out_avals[i].shape)[c]
            for i, k in enumerate(out_names)
        }
        for c in range(N_CORES)
    ]


def kernel(x, w_qkv, w_proj):
    nc = _get_nc()
    in_maps = _make_in_maps(x, w_qkv, w_proj)
    try:
        results = _run_cached(in_maps)
    except Exception:
        res = bass_utils.run_bass_kernel_spmd(nc, in_maps, list(range(N_CORES)))
        results = res.results
    return _reduce_out(results)


# revision 99
# speedup vs baseline: 1.0044x; 1.0044x over previous
"""Causal self-attention (RoPE) Trainium2 kernel — fp8 DoubleRow edition.

Full inputs -> shard across 8 NeuronCores (tensor-parallel over heads x
data-parallel over batch) -> bass/Tile kernel per core -> host partial-sum
unshard.

Reference semantics (B=2, T=2048, C=2048, H=16, hd=128):
    qkv = x @ w_qkv ; q,k,v split ; RoPE(q,k) ; causal softmax attention ;
    y = att @ v ; out = y @ w_proj
Core c handles batch b = c//4 and 4 heads h0 = 4*(c%4). Each core computes
out_partial[T, C] (bf16) = y_heads @ w_proj[rows of its heads]; the host sums
the 4 partials per batch in fp32.

Speed tricks vs the bf16 baseline:
- QKV / V / out projections run as fp8e4m3 DoubleRow matmuls (2 K=128
  products per 0.5-cycle-per-column instruction). Precision is preserved
  with a hi+lo split of both operands (3 of the 4 partial products are
  kept; the dropped lo*lo term is ~0.1%), so each K=128 chunk costs 0.75
  of its bf16 price. Host prepares x_hi/x_lo and w_hi/w_lo (w scaled by
  2^6 so the lo parts stay clear of the e4m3 subnormal floor; all three
  kept products share one power-of-2 psum scale, folded into the drains).
- PV is flipped (p chunk stationary, [v | 1/16] moving) so the softmax
  denominator accumulates for free in psum column 128, eliminating the
  separate ones-matmul rowsum. The 1/16 ones-column folds the 2^4 y
  quantization scale into the reciprocal.
- y [q, d] is normalized via per-partition activation scale, PE-transposed
  in bf16, then quantized to fp8 hi/lo for the DoubleRow out-projection.

Layout trick (baseline): RoPE pairs (2i, 2i+1) are permuted to (i, i+64) by
permuting w_qkv columns on the host, so rotate_half becomes a swap of the
top/bottom 64 partitions on the DVE. S is invariant since the same
orthogonal permutation is applied to q and k.
"""

import sys

sys.path.insert(0, "/opt/trn_rl_repo")

import numpy as np
import ml_dtypes

import concourse.bass as bass
import concourse.mybir as mybir
import concourse.tile as tile
from concourse import bacc, bass_utils

F32 = mybir.dt.float32
BF16 = mybir.dt.bfloat16
FP8 = mybir.dt.float8e4
E4M3 = ml_dtypes.float8_e4m3
DR = mybir.MatmulPerfMode.DoubleRow

T = 2048
C = 2048
HD = 128
NH = 16
NH_LOC = 4          # heads per core
N_CORES = 8
TQ = 512            # q-chunk (moving free dim)
KT = 128            # k-tile (S^T partition dim)
CK = 128            # contraction chunk over C
NCK = C // CK       # 16
NPAIR = NCK // 2    # 8 DoubleRow k-chunk pairs
NTQ = T // TQ       # 4
NKT = T // KT       # 16
SCALE = 1.0 / np.sqrt(HD)
W_SCALE = 64.0      # 2^6 weight prescale (keeps w_lo out of deep subnormals)
Y_SCALE = 16.0      # 2^4 y prescale, folded in via the 1/16 ones column

_compiled_nc = None
DEBUG_DUMPS = False


def _build():
    nc = bacc.Bacc("TRN2", target_bir_lowering=False, debug=False, num_devices=1)

    # pair-major packed layouts: one contiguous DMA per (chunk-pair)
    xq = nc.dram_tensor("xq", [NTQ, NPAIR, CK, 2, 2, TQ], FP8,
                        kind="ExternalInput").ap()
    wqk = nc.dram_tensor("wqk", [NPAIR, CK, 2, 2, 2 * NH_LOC * HD], FP8,
                         kind="ExternalInput").ap()
    wv = nc.dram_tensor("wv", [NPAIR, CK, 2, 2, NH_LOC * HD], FP8,
                        kind="ExternalInput").ap()
    wp = nc.dram_tensor("wp", [HD, 2, NH_LOC, C], FP8, kind="ExternalInput").ap()
    cosP = nc.dram_tensor("cosP", [HD, T], BF16, kind="ExternalInput").ap()
    sinP = nc.dram_tensor("sinP", [HD, T], BF16, kind="ExternalInput").ap()
    masks = nc.dram_tensor("masks", [KT, KT], BF16, kind="ExternalInput").ap()
    ident = nc.dram_tensor("ident", [128, 128], BF16, kind="ExternalInput").ap()
    out = nc.dram_tensor("out", [T, C], BF16, kind="ExternalOutput").ap()
    if DEBUG_DUMPS:
        dq = nc.dram_tensor("dq", [8 * HD, T], BF16, kind="ExternalOutput").ap()
        dv = nc.dram_tensor("dv", [NKT * KT, NH_LOC * (HD + 1)], BF16,
                            kind="ExternalOutput").ap()
        dyt = nc.dram_tensor("dyt", [HD, NKT * 2 * NH_LOC * KT], FP8,
                             kind="ExternalOutput").ap()

    with tile.TileContext(nc) as tc, (
        tc.tile_pool(name="persist", bufs=1)) as persist, (
        tc.tile_pool(name="weights", bufs=1)) as wpool, (
        tc.tile_pool(name="xstream", bufs=2)) as xstream, (
        tc.tile_pool(name="work", bufs=2)) as work, (
        tc.tile_pool(name="ps", bufs=1, space="PSUM")) as psp:

        # ---- persistent SBUF ----
        cos_sb = persist.tile([HD, T], BF16, tag="cos")
        sin_sb = persist.tile([HD, T], BF16, tag="sin")
        mask01_sb = persist.tile([KT, KT], BF16, tag="mask")
        id_sb = persist.tile([128, 128], BF16, tag="ident")

        # Q^T/K^T (d-major), V+ones (token-major), y^T fp8 hi/lo staging
        qk_sb = [persist.tile([HD, T], BF16, name=f"qk{m}", tag=f"qk{m}") for m in range(8)]
        v_sb = [persist.tile([KT, NH_LOC, HD + 1], BF16, name=f"v{i}", tag=f"v{i}") for i in range(NKT)]
        # [d, tok-tile, {hi,lo}, head, tok] — slot dim pairs either heads
        # (stride 128, at fixed hl) or hi/lo of one head (stride 512)
        yt_sb = persist.tile([HD, NKT, 2, NH_LOC, KT], FP8, tag="yt")
        for i in range(NKT):
            nc.vector.memset(v_sb[i][:, :, HD:HD + 1], 1.0 / Y_SCALE)

        # ---- weights (whole kernel lifetime) ----
        # pair tiles: [128, {kc even, kc odd}, {hi, lo}, cols]
        wqk_p = [wpool.tile([CK, 2, 2, 2 * NH_LOC * HD], FP8, name=f"wqk{i}", tag=f"wqk{i}") for i in range(NPAIR)]
        wv_p = [wpool.tile([CK, 2, 2, NH_LOC * HD], FP8, name=f"wv{i}", tag=f"wv{i}") for i in range(NPAIR)]
        # [d, {lo, hi}, head, c-cols]
        wp_sb = wpool.tile([HD, 2, NH_LOC, C], FP8, tag="wp")
        # x pair tiles: [128, {kc even, kc odd}, {lo, hi}, tq]
        xc0 = [xstream.tile([CK, 2, 2, TQ], FP8, name=f"xp{i}", tag=f"xp{i}") for i in range(NPAIR)]
        # Startup DMAs on the SP HWDGE queue in consumption order
        for i in range(NPAIR):
            nc.sync.dma_start(wqk_p[i][:], wqk[i])
            nc.sync.dma_start(xc0[i][:], xq[0, i])
        nc.sync.dma_start(cos_sb[:], cosP)
        nc.sync.dma_start(sin_sb[:], sinP)
        for i in range(NPAIR):
            nc.sync.dma_start(wv_p[i][:], wv[i])
        nc.sync.dma_start(mask01_sb[:], masks)
        nc.sync.dma_start(id_sb[:], ident)
        nc.sync.dma_start(wp_sb[:], wp)

        # PSUM tags (8 banks total, shared across phases via slot cycling).
        # NOTE: start=True zeroes the WHOLE bank (pending-zero is
        # bank-granular), so every accumulation group owns a full bank:
        #   "a": QKV psums m=0..3 / S^T tiles + y transposes / V psums /
        #        out-proj psums                                           (4)
        #   "b": QKV psums m=4,5 / flipped-PV accumulators s=0,1          (2)
        #   "c": QKV psum m=6 / flipped-PV accumulator s=2                (1)
        #   "d": QKV psum m=7 / flipped-PV accumulator s=3                (1)
        def ps_a():
            return psp.tile([128, TQ], F32, name="psa", tag="a", bufs=4)
        def ps_b():
            return psp.tile([128, TQ], F32, name="psb", tag="b", bufs=2)
        def ps_c():
            return psp.tile([128, TQ], F32, name="psc", tag="c", bufs=1)
        def ps_d():
            return psp.tile([128, TQ], F32, name="psd", tag="d", bufs=1)
        def ps_a_t():
            # transpose staging; same banks as "a" (2KB/partition each)
            return psp.tile([128, 1024], BF16, name="pst", tag="a", bufs=4)

        # warmup: keep the PE busy through the startup DMA window so the
        # p-state ramp (2.4GHz after 3us continuous busy) is paid for free.
        # memset on the (idle) GPSIMD so the DVE's v-ones memsets don't
        # delay the first warmup matmul.
        warm = work.tile([128, TQ], BF16, tag="warm", bufs=1)
        nc.gpsimd.memset(warm[:], 0.0)
        wps = psp.tile([128, TQ], F32, name="wps", tag="a", bufs=4)
        for _ in range(14):
            nc.tensor.matmul(wps[0:16, 0:256], warm[:, 0:16], warm[:, 0:256],
                             start=True, stop=True)

        def qkv_phase(jq, xp, skip=()):
            """Hi-lo fp8 DoubleRow QKV + V projection for token chunk jq
            (pair-outer so all psums progress as x/w tiles land), plus
            next-x prefetch. Products carry scale 2^6 (from w); drains
            scale by 2^-6."""
            tsl = slice(jq * TQ, (jq + 1) * TQ)
            mk = [ps_a] * 4 + [ps_b] * 2 + [ps_c, ps_d]
            qk_ps = [None if m in skip else mk[m]() for m in range(8)]
            # cross terms first within each pair: cross(2i) only needs
            # chunk 2i's DMAs, so the PE can start one transfer earlier
            for i in range(NPAIR):
                # cross: w_hi x x_lo + w_lo x x_hi, per k-chunk
                for j in range(2):
                    for m in range(8):
                        if qk_ps[m] is None:
                            continue
                        nc.tensor.matmul(
                            qk_ps[m][:],
                            wqk_p[i][:, j, :, m * 128:(m + 1) * 128],
                            xp[i][:, j, :, :],
                            start=(i == 0 and j == 0), stop=False, perf_mode=DR,
                        )
                # term1: w_hi (pair) x x_hi (pair)
                last = (i == NPAIR - 1)
                for m in range(8):
                    if qk_ps[m] is None:
                        continue
                    nc.tensor.matmul(
                        qk_ps[m][:],
                        wqk_p[i][:, :, 0, m * 128:(m + 1) * 128],
                        xp[i][:, :, 1, :],
                        start=False, stop=last, perf_mode=DR,
                    )
            # drain q/k of head h adjacently (0,4 then 1,5 ...) so the
            # attention phase's first S matmuls unblock after two drains
            for m in (0, 4, 1, 5, 2, 6, 3, 7):
                if qk_ps[m] is None:
                    continue
                dst = qk_sb[m][:, tsl]
                nc.scalar.activation(dst, qk_ps[m][:],
                                     mybir.ActivationFunctionType.Copy,
                                     scale=1.0 / W_SCALE)
                # RoPE in place on DVE: tmp = swap(dst)*sinSwap computed with
                # partition-aligned inputs and a shifted output partition.
                tmp = work.tile([HD, TQ], BF16, tag="rope", bufs=3)
                nc.vector.tensor_mul(tmp[0:64, :], dst[64:128, :], sin_sb[64:128, tsl])
                nc.vector.tensor_mul(tmp[64:128, :], dst[0:64, :], sin_sb[0:64, tsl])
                nc.vector.tensor_mul(dst, dst, cos_sb[:, tsl])
                nc.vector.tensor_add(dst, dst, tmp[:])

            # V projection, s4-outer: each token-subtile psum completes
            # and drains early, freeing its "a" slot before the phase
            # boundary (the pair-major packed wv DMAs land well before the
            # jq0 V phase starts, so this is safe for jq0 too).
            if True:
                for s4 in range(4):
                    # last subtile goes to bank "d" so its late drain does
                    # not gate the attention phase's first S matmul
                    v_ps = (psp.tile([128, 4, KT], F32, name="psv", tag="d",
                                     bufs=1) if s4 == 3 else
                            psp.tile([128, 4, KT], F32, name="psv", tag="a",
                                     bufs=4))
                    for i in range(NPAIR):
                        for j in range(2):
                            nc.tensor.matmul(
                                v_ps[:],
                                xp[i][:, j, :, s4 * KT:(s4 + 1) * KT],
                                wv_p[i][:, j, :, :],
                                start=(i == 0 and j == 0), stop=False,
                                perf_mode=DR,
                            )
                        nc.tensor.matmul(
                            v_ps[:],
                            xp[i][:, :, 1, s4 * KT:(s4 + 1) * KT],
                            wv_p[i][:, :, 0, :],
                            start=False, stop=(i == NPAIR - 1), perf_mode=DR,
                        )
                    nc.scalar.activation(
                        v_sb[jq * 4 + s4][:, :, 0:HD], v_ps[:],
                        mybir.ActivationFunctionType.Copy,
                        scale=1.0 / W_SCALE)

            # prefetch next x chunk right after its predecessor is consumed
            if jq + 1 < NTQ:
                xp_next = [xstream.tile([CK, 2, 2, TQ], FP8, name=f"xp{i}", tag=f"xp{i}") for i in range(NPAIR)]
                for i in range(NPAIR):
                    nc.sync.dma_start(xp_next[i][:], xq[jq + 1, i])
            else:
                xp_next = None
            return xp_next

        # out-projection tile: 6 DoubleRow matmuls + drain + DMA, split in
        # two filler-granular halves sharing one psum group.
        # products carry scale 2^4 (y) * 2^6 (wp) -> drain by 2^-10
        def op_tile_thunks(jqo, s4, cc, use_act, tail=False, split=False):
            it = jqo * 4 + s4
            csl = slice(cc * TQ, (cc + 1) * TQ)
            st = {}

            def part1():
                ps = st["ps"] = ps_a()
                for hp in range(2):
                    nc.tensor.matmul(
                        ps[:],
                        yt_sb[:, it, 0, 2 * hp:2 * hp + 2, :],
                        wp_sb[:, 1, 2 * hp:2 * hp + 2, csl],
                        start=(hp == 0), stop=False, perf_mode=DR,
                    )
                nc.tensor.matmul(
                    ps[:], yt_sb[:, it, :, 0, :], wp_sb[:, :, 0, csl],
                    start=False, stop=False, perf_mode=DR,
                )

            def part2():
                ps = st["ps"]
                for h2 in range(1, NH_LOC):
                    nc.tensor.matmul(
                        ps[:],
                        yt_sb[:, it, :, h2, :],
                        wp_sb[:, :, h2, csl],
                        start=False, stop=(h2 == NH_LOC - 1), perf_mode=DR,
                    )
                ot = work.tile([128, TQ], BF16, tag="ot", bufs=8)
                if tail:
                    # final tiles: fan the drains and DMA issues across all
                    # engine queues to shrink the end-of-kernel flush
                    if cc % 2 == 0:
                        nc.scalar.activation(
                            ot[:], ps[:], mybir.ActivationFunctionType.Copy,
                            scale=1.0 / (W_SCALE * Y_SCALE),
                        )
                    else:
                        nc.vector.tensor_scalar_mul(
                            ot[:], ps[:], 1.0 / (W_SCALE * Y_SCALE))
                    dma_eng = [nc.scalar, nc.sync, nc.gpsimd, nc.sync][cc % 4]
                    dma_eng.dma_start(out[it * KT:(it + 1) * KT, csl], ot[:])
                elif use_act:
                    nc.scalar.activation(
                        ot[:], ps[:], mybir.ActivationFunctionType.Copy,
                        scale=1.0 / (W_SCALE * Y_SCALE),
                    )
                    nc.scalar.dma_start(out[it * KT:(it + 1) * KT, csl], ot[:])
                else:
                    nc.vector.tensor_scalar_mul(
                        ot[:], ps[:], 1.0 / (W_SCALE * Y_SCALE))
                    nc.sync.dma_start(out[it * KT:(it + 1) * KT, csl], ot[:])

            return [part1, part2]

        def emit_op_tile(jqo, s4, cc, use_act, tail=False, split=False):
            for t in op_tile_thunks(jqo, s4, cc, use_act, tail=tail,
                                    split=split):
                t()

        # out-proj(jq) tiles are emitted as FILLER inside attention(jq+1):
        # per-tile exp on the activation engine outpaces the PE's S+PV work
        # there, so the PE interleaves independent out-proj matmuls into
        # the exp-serialization gaps.
        pending_ops = []

        def pull_ops(n=1):
            for _ in range(n):
                if pending_ops:
                    pending_ops.pop(0)()

        def qkv_m_thunks(jq1, xp1, m):
            """Deferred QKV m-group of chunk jq1, as filler thunks."""
            tsl1 = slice(jq1 * TQ, (jq1 + 1) * TQ)
            mcols = slice(m * 128, (m + 1) * 128)
            st = {}

            def pairs(i0, i1, first):
                ps = st["ps"]
                for i in range(i0, i1):
                    for j in range(2):
                        nc.tensor.matmul(
                            ps[:], wqk_p[i][:, j, :, mcols], xp1[i][:, j, :, :],
                            start=(first and i == i0 and j == 0), stop=False,
                            perf_mode=DR)
                    nc.tensor.matmul(
                        ps[:], wqk_p[i][:, :, 0, mcols], xp1[i][:, :, 1, :],
                        start=False, stop=(i == NPAIR - 1), perf_mode=DR)

            def t1():
                st["ps"] = ps_a()
                pairs(0, 3, True)

            def t2():
                pairs(3, 6, False)

            def t3():
                pairs(6, NPAIR, False)

            def t4():
                dst = qk_sb[m][:, tsl1]
                nc.scalar.activation(dst, st["ps"][:],
                                     mybir.ActivationFunctionType.Copy,
                                     scale=1.0 / W_SCALE)
                tmp = work.tile([HD, TQ], BF16, tag="rope", bufs=3)
                nc.vector.tensor_mul(tmp[0:64, :], dst[64:128, :], sin_sb[64:128, tsl1])
                nc.vector.tensor_mul(tmp[64:128, :], dst[0:64, :], sin_sb[0:64, tsl1])
                nc.vector.tensor_mul(dst, dst, cos_sb[:, tsl1])
                nc.vector.tensor_add(dst, dst, tmp[:])

            return [t1, t2, t3, t4]

        xp_sb = xc0
        for jq in range(NTQ):
            tsl = slice(jq * TQ, (jq + 1) * TQ)

            xp_next = qkv_phase(jq, xp_sb, skip=((7,) if jq == 1 else ()))
            if jq == 0:
                # attention(0) has no out-proj filler yet; feed it the
                # deferred m=7 QKV group of chunk 1
                pending_ops = qkv_m_thunks(1, xp_next, 7)

            # ======== attention for q-chunk jq, all local heads ========
            nk = 4 * jq + 4
            pull_every = max(1, (4 * nk) // 18)
            pv_count = 0
            pull_ops(6)  # cover the qk-drain/RoPE wait before the first S
            tp_tag = [("b", 2), ("b", 2), ("c", 1), ("d", 1)]
            for h in range(NH_LOC):
                # flipped-PV accumulators: [q, {y cols 0..127, rowsum/16}],
                # one full psum bank per accumulation group
                pv_ps = [
                    psp.tile([128, HD + 1], F32, name="pvb0", tag="b", bufs=2),
                    psp.tile([128, HD + 1], F32, name="pvb1", tag="b", bufs=2),
                    psp.tile([128, HD + 1], F32, name="pvc", tag="c", bufs=1),
                    psp.tile([128, HD + 1], F32, name="pvd", tag="d", bufs=1),
                ]

                # software-pipelined by one tile: S(ik+1) is issued before
                # PV(ik) so the PE has work while exp(ik) runs
                def s_exp(ik):
                    r = ik - 4 * jq
                    # columns q < 128*r of this S^T tile are fully masked
                    qo = 128 * r if r >= 1 else 0
                    diag = r >= 0
                    s_ps = ps_a()
                    nc.tensor.matmul(
                        s_ps[:, qo:],
                        qk_sb[4 + h][:, ik * KT:(ik + 1) * KT],
                        qk_sb[h][:, jq * TQ + qo:(jq + 1) * TQ],
                        start=True,
                        stop=not diag,
                    )
                    if diag:
                        # causal mask folded into the logits on the PE:
                        # adds -1e5 where k > q in the straddling 128-block
                        # (maskM.T @ I), so exp underflows to exact zeros and
                        # no masking sits on the exp->PV chain
                        nc.tensor.matmul(
                            s_ps[:, qo:qo + KT], mask01_sb[:], id_sb[:],
                            start=False, stop=True,
                        )
                    p_t = work.tile([KT, TQ], BF16, tag="p", bufs=8)
                    nc.scalar.activation(
                        p_t[:, qo:], s_ps[:, qo:],
                        mybir.ActivationFunctionType.Exp,
                        scale=float(SCALE),
                    )
                    return p_t, qo

                rsum = work.tile([128, 4], F32, tag="rsum", bufs=2)

                def pv(ik, p_t, qo):
                    r = ik - 4 * jq
                    ss = list(range(max(r, 0), 4))
                    for s in ss:
                        s_abs = 4 * jq + s
                        nc.tensor.matmul(
                            pv_ps[s][:],
                            p_t[:, s * KT:(s + 1) * KT],
                            v_sb[ik][:, h, :],
                            start=(ik == 0),
                            stop=(ik == s_abs),
                        )
                        if ik == s_abs:
                            # stage the rowsum copy as soon as this
                            # subtile's accumulation closes, ahead of the
                            # recip -> normalize -> transpose chain
                            nc.vector.tensor_copy(
                                rsum[:, s:s + 1], pv_ps[s][:, HD:HD + 1])

                def maybe_pull():
                    nonlocal pv_count
                    pv_count += 1
                    if pv_count % pull_every == 0:
                        pull_ops(1)

                # depth-2 software pipeline: two S/exp tiles in flight so
                # the exp ack/semaphore latency never reaches the PE
                depth = min(2, nk)
                pend = [s_exp(ik) for ik in range(depth)]
                for ik in range(depth, nk):
                    nxt = s_exp(ik)
                    # filler goes BEFORE pv: pv blocks the in-order PE
                    # queue on exp, filler does not
                    maybe_pull()
                    pv(ik - depth, *pend[0])
                    pend = pend[1:] + [nxt]
                for z, ik in enumerate(range(nk - depth, nk)):
                    maybe_pull()
                    pv(ik, *pend[z])

                # normalize (x16) on DVE, PE-transpose via the freed PV
                # banks, quantize hi/lo fp8 on DVE (act keeps only exp)
                recip = work.tile([128, 4], F32, tag="recip", bufs=2)
                nc.vector.reciprocal_approx_fast(recip[:], rsum[:])
                yns = []
                for s in range(4):
                    yn = work.tile([128, KT], BF16, tag="yn", bufs=4)
                    nc.vector.tensor_scalar(
                        yn[:], pv_ps[s][:, 0:HD], recip[:, s:s + 1], None,
                        mybir.AluOpType.mult)
                    yns.append(yn)
                pull_ops(2)  # cover the recip/normalize latency
                for s in range(4):
                    it = jq * 4 + s
                    tag, nb = tp_tag[s]
                    tp = psp.tile([128, 1024], BF16, name="tp", tag=tag, bufs=nb)
                    nc.tensor.transpose(tp[:, 0:KT], yns[s][:], id_sb[:])
                    nc.vector.tensor_copy(yt_sb[:, it, 0, h, :], tp[:, 0:KT])
                    nc.vector.tensor_tensor(yt_sb[:, it, 1, h, :], tp[:, 0:KT],
                                            yt_sb[:, it, 0, h, :],
                                            mybir.AluOpType.subtract)
                    if jq == NTQ - 1 and h == NH_LOC - 1:
                        # final chunk: out-proj inline as soon as the last
                        # head's subtile lands (shrinks the DMA tail)
                        for cc in range(4):
                            emit_op_tile(jq, s, cc, use_act=(cc % 2 == 0),
                                         tail=True)

            pull_ops(32)  # flush any leftover out-proj tiles of jq-1

            if jq < NTQ - 1:
                pending_ops = [
                    t
                    for s4 in range(4) for cc in range(4)
                    for t in op_tile_thunks(jq, s4, cc, use_act=False)
                ]

            xp_sb = xp_next

        if DEBUG_DUMPS:
            for m in range(8):
                nc.sync.dma_start(dq[m * HD:(m + 1) * HD, :], qk_sb[m][:])
            for i in range(NKT):
                nc.sync.dma_start(dv[i * KT:(i + 1) * KT, :], v_sb[i][:])
            nc.sync.dma_start(dyt, yt_sb[:])

    nc.compile()
    return nc


def _get_nc():
    global _compiled_nc
    if _compiled_nc is None:
        _compiled_nc = _build()
    return _compiled_nc


def _rope_tables():
    t = np.arange(T, dtype=np.float64)
    inv_freq = 1.0 / (10000.0 ** (np.arange(0, HD, 2, dtype=np.float64) / HD))
    freqs = np.outer(t, inv_freq)            # [T, 64]
    cos_half = np.cos(freqs).T               # [64, T]
    sin_half = np.sin(freqs).T
    cosP = np.concatenate([cos_half, cos_half], axis=0)      # [128, T]
    # tmp[j]    = dst[j+64] * sinSwap[j+64]  (= -sin_half[j])
    # tmp[j+64] = dst[j]    * sinSwap[j]     (= +sin_half[j])
    sinSwap = np.concatenate([sin_half, -sin_half], axis=0)
    return (cosP.astype(ml_dtypes.bfloat16), sinSwap.astype(ml_dtypes.bfloat16))


def _mask_tiles():
    # maskM[j, k] = -1e5 where k > j: (maskM.T @ I)[k, q] masks k > q
    j = np.arange(KT)[:, None]               # [128, 1]
    k = np.arange(KT)[None, :]               # [1, 128]
    return np.where(k > j, -1.0e5, 0.0).astype(ml_dtypes.bfloat16)


def _head_perm(h0):
    """Permuted q/k columns for heads h0..h0+3: pairs (2i,2i+1)->(i,i+64)."""
    cols = []
    for h in range(h0, h0 + NH_LOC):
        base = h * HD
        cols.extend(base + 2 * np.arange(64))
        cols.extend(base + 2 * np.arange(64) + 1)
    return np.array(cols)


def _hilo(a, scale):
    """(hi, lo) e4m3 split of a*scale (lo captures the rounding residual)."""
    s = (np.asarray(a, dtype=np.float32) * np.float32(scale))
    hi = s.astype(E4M3)
    lo = (s - hi.astype(np.float32)).astype(E4M3)
    return hi, lo


def _make_in_maps(x, w_qkv, w_proj):
    x = np.asarray(x)
    w_qkv = np.asarray(w_qkv)
    w_proj = np.asarray(w_proj)
    B = x.shape[0]
    assert x.shape == (B, T, C) and B == 2

    cosP, sinP = _rope_tables()
    masks = _mask_tiles()
    ident = np.eye(128, dtype=ml_dtypes.bfloat16)

    xq_b = []
    for b in range(B):
        xT = np.ascontiguousarray(x[b].T)
        hi, lo = _hilo(xT, 1.0)
        xs = np.stack([lo, hi], axis=1)            # [C, 2, T]
        # -> [NTQ, NPAIR, 128, {kc in pair}, {lo,hi}, TQ] pair-major pack
        xs = xs.reshape(NPAIR, 2, CK, 2, NTQ, TQ).transpose(4, 0, 2, 1, 3, 5)
        xq_b.append(np.ascontiguousarray(xs))

    in_maps = []
    for c in range(N_CORES):
        b = c // 4
        h0 = NH_LOC * (c % 4)
        perm = _head_perm(h0)
        def _pack_pairs(a):
            # [C, 2, W] -> [NPAIR, 128, {kc in pair}, {hi,lo}, W]
            W = a.shape[-1]
            return np.ascontiguousarray(
                a.reshape(NPAIR, 2, CK, 2, W).transpose(0, 2, 1, 3, 4))

        wqk_c = np.concatenate(
            [w_qkv[:, perm], w_qkv[:, C + perm]], axis=1
        )                                                   # [C, 1024]
        hi, lo = _hilo(wqk_c, W_SCALE)
        wqk_8 = _pack_pairs(np.stack([hi, lo], axis=1))
        vcols = np.arange(h0 * HD, (h0 + NH_LOC) * HD)
        hi, lo = _hilo(w_qkv[:, 2 * C + vcols], W_SCALE)
        wv_8 = _pack_pairs(np.stack([hi, lo], axis=1))
        wp_c = w_proj[h0 * HD:(h0 + NH_LOC) * HD, :]              # [512, C]
        wp_d = wp_c.reshape(NH_LOC, HD, C).transpose(1, 0, 2)     # [128,4,C]
        hi, lo = _hilo(wp_d, W_SCALE)
        wp_8 = np.ascontiguousarray(np.stack([lo, hi], axis=1))   # [128,2,4,C]
        in_maps.append({
            "xq": xq_b[b],
            "wqk": wqk_8,
            "wv": wv_8,
            "wp": wp_8,
            "cosP": cosP,
            "sinP": sinP,
            "masks": masks,
            "ident": ident,
        })
    return in_maps


def _reduce_out(results):
    out = np.zeros((2, T, C), dtype=np.float32)
    for c in range(N_CORES):
        out[c // 4] += results[c]["out"].astype(np.float32)
    return out


_cached_exec = None


def _get_cached_exec():
    """Build (once) a jitted SPMD executable for the compiled Bass module."""
    global _cached_exec
    if _cached_exec is not None:
        return _cached_exec
    import jax
    from jax.experimental.shard_map import shard_map
    from jax.sharding import Mesh, PartitionSpec
    from concourse import bass2jax

    nc = _get_nc()
    bass2jax.install_neuronx_cc_hook()
    partition_name = nc.partition_id_tensor.name if nc.partition_id_tensor else None
    in_names, out_names, out_avals = [], [], []
    for alloc in nc.m.functions[0].allocations:
        if not isinstance(alloc, mybir.MemoryLocationSet):
            continue
        name = alloc.memorylocations[0].name
        if alloc.kind == "ExternalInput":
            if name != partition_name:
                in_names.append(name)
        elif alloc.kind == "ExternalOutput":
            out_names.append(name)
            out_avals.append(
                jax.core.ShapedArray(
                    tuple(alloc.tensor_shape), mybir.dt.np(alloc.dtype)
                )
            )
    n_params = len(in_names)
    all_names = (
        tuple(in_names) + tuple(out_names)
        + ((partition_name,) if partition_name else ())
    )
    donate = tuple(range(n_params, n_params + len(out_names)))

    def _body(*args):
        operands = list(args)
        if partition_name is not None:
            operands.append(bass2jax.partition_id_tensor())
        outs = bass2jax._bass_exec_p.bind(
            *operands,
            out_avals=tuple(out_avals),
            in_names=all_names,
            out_names=tuple(out_names),
            lowering_input_output_aliases=(),
            sim_require_finite=True,
            sim_require_nnan=True,
            nc=nc,
        )
        return tuple(outs)

    devices = jax.devices()[:N_CORES]
    mesh = Mesh(np.asarray(devices), ("core",))
    nin = n_params + len(out_names)
    sharded = jax.jit(
        shard_map(
            _body,
            mesh=mesh,
            in_specs=(PartitionSpec("core"),) * nin,
            out_specs=(PartitionSpec("core"),) * len(out_names),
            check_rep=False,
        ),
        donate_argnums=donate,
        keep_unused=True,
    )
    _cached_exec = (sharded, in_names, out_names, out_avals)
    return _cached_exec


def _run_cached(in_maps):
    sharded, in_names, out_names, out_avals = _get_cached_exec()
    concat_in = [
        np.concatenate([np.asarray(in_maps[c][k]) for c in range(N_CORES)], axis=0)
        for k in in_names
    ]
    concat_zeros = [
        np.zeros((N_CORES * av.shape[0], *av.shape[1:]), av.dtype)
        for av in out_avals
    ]
    out_arrs = sharded(*concat_in, *concat_zeros)
    return [
        {
            k: np.asarray(out_arrs[i]).reshape(N_CORES, *out_avals[i].shape)[c]
            for i, k in enumerate(out_names)
        }
        for c in range(N_CORES)
    ]


def kernel(x, w_qkv, w_proj):
    nc = _get_nc()
    in_maps = _make_in_maps(x, w_qkv, w_proj)
    try:
        results = _run_cached(in_maps)
    except Exception:
        res = bass_utils.run_bass_kernel_spmd(nc, in_maps, list(range(N_CORES)))
        results = res.results
    return _reduce_out(results)
